# revision 4
# baseline (speedup 1.0000x reference)
"""3-layer GAT (GATConv) network on 8 Trainium2 NeuronCores.

Strategy (edge-parallel, dst-sharded):
  - Nodes are sharded evenly: core c owns nodes [c*1250, (c+1)*1250), padded
    to NP=1280 (10 tiles of 128).  Edges are sorted by dst and assigned to the
    core owning their dst node, grouped per 128-node dst tile, padded to a
    per-tile chunk count CH[t] shared across cores (same NEFF everywhere).
  - Per layer: each core computes the dense transforms for its own node shard
    (fused matmul [W | Wl | W@As | W@Ad]), packs a per-node "table row"
    [Hfeat bf16 | ALs fp32 | ALd fp32] and AllGathers the table.  The edge
    phase gathers table rows by src (indirect DMA), builds per-edge softmax
    numerators ex = exp(leaky_relu(ALs[src]+ALd[dst]+ALe)), scales the
    gathered features by ex, and aggregates per dst tile with a one-hot
    mask matmul (also producing the softmax denominators).  Output is
    normalized, skip+bias added, ELU applied, and becomes the next layer's
    input.  No inter-core traffic except the 3 AllGathers.
"""
import functools

import numpy as np
import ml_dtypes

import concourse.bass as bass
import concourse.bacc as bacc
import concourse.tile as tile
from concourse import mybir
from concourse.bass_utils import run_bass_kernel_spmd

P = 128
NCORES = 8
N = 10000
E = 160000
NSH = N // NCORES          # 1250 real nodes per core
NT = 10                    # dst tiles per core
NP = NT * P                # padded nodes per core (1280)
F_IN = 512
D12 = 1024                 # hidden width layers 1-2
H12, C12 = 4, 256
D3, H3, C3 = 96, 6, 16
NCLS = 16
RW12 = D12 + 4 * H12       # table row lanes (bf16) layers 1/2: 1024+16
RW3 = 128                  # layer 3 row: 96 feat + 12 ALs + 12 ALd + 8 pad
ALS_OFF12 = D12            # bf16 lane offset of ALs (fp32-bitcast) in row
ALD_OFF12 = D12 + 2 * H12
ALS_OFF3 = D3
ALD_OFF3 = D3 + 2 * H3

f32 = mybir.dt.float32
bf16 = mybir.dt.bfloat16
i32 = mybir.dt.int32
u8 = mybir.dt.uint8
BF = ml_dtypes.bfloat16


# ----------------------------------------------------------------- device code

def _dense_phase(nc, tc, ctx, *, K, wdram, vesb, brep_dram, lhs_tiles,
                 skip_sb, tbl_agin, Dmain, Dskip, HH, als_off):
    """Dense transforms for own node shard.  Writes table rows (Hfeat+AL)
    into tbl_agin and skip+bias into skip_sb ([P, NT*Dskip] fp32)."""
    Wcols = Dmain + Dskip + 2 * HH
    ms = Dmain + Dskip               # main+skip region
    nj = (ms + 511) // 512
    import contextlib
    with contextlib.ExitStack() as st:
        wpool = st.enter_context(tc.tile_pool(name="wp", bufs=K))
        pspool = st.enter_context(tc.tile_pool(name="dps", bufs=4, space="PSUM"))
        alpool = st.enter_context(tc.tile_pool(name="dal", bufs=2, space="PSUM"))
        tpool = st.enter_context(tc.tile_pool(name="dtab", bufs=3))
        bpool = st.enter_context(tc.tile_pool(name="dbr", bufs=1))

        brep = bpool.tile([P, Dskip], f32)
        nc.sync.dma_start(brep[:], brep_dram[:])
        wsb = []
        for k in range(K):
            w = wpool.tile([P, Wcols], bf16, tag="w")
            nc.sync.dma_start(w[:], wdram[k * P:(k + 1) * P, :])
            wsb.append(w)

        for t in range(NT):
            ps = []
            for j in range(nj):
                width = min(512, ms - j * 512)
                ps.append(pspool.tile([P, width], f32, space="PSUM", tag="m", name=f"dps{j}"))
            psal = alpool.tile([P, 2 * HH], f32, space="PSUM", tag="al")
            for k in range(K):
                lhsT = lhs_tiles[k][:, t * P:(t + 1) * P]
                first, last = (k == 0), (k == K - 1)
                for j in range(nj):
                    width = min(512, ms - j * 512)
                    nc.tensor.matmul(ps[j][:], lhsT=lhsT,
                                     rhs=wsb[k][:, j * 512:j * 512 + width],
                                     start=first, stop=last)
                nc.tensor.matmul(psal[:], lhsT=lhsT,
                                 rhs=wsb[k][:, ms:ms + 2 * HH],
                                 start=first, stop=last)
            # pack table row: Hfeat bf16 + AL fp32 (bitcast into bf16 lanes)
            rw = als_off + 4 * HH if als_off + 4 * HH == RW12 else RW3
            tabt = tpool.tile([P, rw], bf16, tag="t")
            done = 0
            while done < Dmain:
                j, off = done // 512, done % 512
                width = min(512 - off, Dmain - done)
                nc.vector.tensor_copy(tabt[:, done:done + width],
                                      ps[j][:, off:off + width])
                done += width
            nc.vector.tensor_copy(
                tabt[:, als_off:als_off + 4 * HH].bitcast(f32), psal[:])
            if rw > als_off + 4 * HH:  # zero pad lanes (layer 3)
                nc.gpsimd.memset(tabt[:, als_off + 4 * HH:], 0.0)
            nc.sync.dma_start(tbl_agin[t * P:(t + 1) * P, :], tabt[:])
            # skip + bias -> persistent sbuf
            done = 0
            while done < Dskip:
                j, off = (Dmain + done) // 512, (Dmain + done) % 512
                width = min(512 - off, Dskip - done)
                nc.vector.tensor_tensor(
                    out=skip_sb[:, t * Dskip + done:t * Dskip + done + width],
                    in0=ps[j][:, off:off + width],
                    in1=brep[:, done:done + width],
                    op=mybir.AluOpType.add)
                done += width


def _edge_phase(nc, tc, ctx, CH, *, tbl, vesb, eat_sb, srcg_sb, dstg_sb,
                dstl_sb, iota_row, neg1, skip_sb, Dmain, HH, CC, als_off,
                ald_off, out_writer):
    """Edge gather + softmax + aggregation.  out_writer(t, pre_ap, pool)
    consumes the per-dst-tile normalized gat+skip fp32 tile."""
    import contextlib
    rw = als_off + 4 * HH if als_off + 4 * HH == RW12 else RW3
    gcols = Dmain + HH  # scaled feats + ex
    with contextlib.ExitStack() as st:
        gpool = st.enter_context(tc.tile_pool(name="eg", bufs=4))
        gppool = st.enter_context(tc.tile_pool(name="egp", bufs=4))
        spool = st.enter_context(tc.tile_pool(name="esm", bufs=6))
        mpool = st.enter_context(tc.tile_pool(name="emask", bufs=4))
        apool = st.enter_context(tc.tile_pool(name="eagg", bufs=4, space="PSUM"))
        epool = st.enter_context(tc.tile_pool(name="eale", bufs=2, space="PSUM"))
        opool = st.enter_context(tc.tile_pool(name="eout", bufs=2))

        g = 0
        for t in range(NT):
            nja = (Dmain + 511) // 512
            agg = []
            for j in range(nja):
                width = min(512, Dmain - j * 512)
                agg.append(apool.tile([P, width], f32, space="PSUM", tag="agg", name=f"agg{j}"))
            den = epool.tile([P, HH], f32, space="PSUM", tag="den")
            for ch in range(CH[t]):
                first, last = (ch == 0), (ch == CH[t] - 1)
                # gather full rows by src
                G = gpool.tile([P, rw], bf16, tag="G")
                nc.gpsimd.indirect_dma_start(
                    out=G[:], out_offset=None, in_=tbl[:],
                    in_offset=bass.IndirectOffsetOnAxis(
                        ap=srcg_sb[:, g:g + 1], axis=0))
                # gather ALd (by dst)
                tail = gpool.tile([P, 2 * HH], bf16, tag="tail")
                nc.gpsimd.indirect_dma_start(
                    out=tail[:], out_offset=None, in_=tbl[:],
                    in_offset=bass.IndirectOffsetOnAxis(
                        ap=dstg_sb[:, g:g + 1], axis=0),
                    element_offset=ald_off)
                # ALe = eattr @ ve
                psale = epool.tile([P, HH], f32, space="PSUM", tag="ale")
                nc.tensor.matmul(psale[:], lhsT=eat_sb[:, g * P:(g + 1) * P],
                                 rhs=vesb[:], start=True, stop=True)
                # logits -> leaky -> exp
                lg = spool.tile([P, HH], f32, tag="lg")
                nc.vector.tensor_tensor(
                    out=lg[:], in0=G[:, als_off:als_off + 2 * HH].bitcast(f32),
                    in1=tail[:].bitcast(f32), op=mybir.AluOpType.add)
                nc.vector.tensor_tensor(out=lg[:], in0=lg[:], in1=psale[:],
                                        op=mybir.AluOpType.add)
                lk = spool.tile([P, HH], f32, tag="lk")
                nc.vector.tensor_scalar_mul(lk[:], lg[:], 0.2)
                nc.vector.tensor_tensor(out=lk[:], in0=lg[:], in1=lk[:],
                                        op=mybir.AluOpType.max)
                ex = spool.tile([P, HH], f32, tag="ex")
                nc.scalar.activation(ex[:], lk[:],
                                     mybir.ActivationFunctionType.Exp)
                # mask
                mask = mpool.tile([P, P], bf16, tag="mk")
                nc.vector.tensor_tensor(
                    out=mask[:], in0=dstl_sb[:, g:g + 1].to_broadcast([P, P]),
                    in1=iota_row[:], op=mybir.AluOpType.is_equal)
                # scaled features + ex column block
                Gp = gppool.tile([P, gcols], bf16, tag="Gp")
                for h in range(HH):
                    nc.vector.tensor_scalar_mul(
                        Gp[:, h * CC:(h + 1) * CC], G[:, h * CC:(h + 1) * CC],
                        ex[:, h:h + 1])
                nc.scalar.activation(Gp[:, Dmain:Dmain + HH], ex[:],
                                     mybir.ActivationFunctionType.Copy)
                # aggregate
                for j in range(nja):
                    width = min(512, Dmain - j * 512)
                    nc.tensor.matmul(agg[j][:], lhsT=mask[:],
                                     rhs=Gp[:, j * 512:j * 512 + width],
                                     start=first, stop=last)
                nc.tensor.matmul(den[:], lhsT=mask[:],
                                 rhs=Gp[:, Dmain:Dmain + HH],
                                 start=first, stop=last)
                g += 1
            # normalize + skip
            deni = spool.tile([P, HH], f32, tag="deni")
            nc.vector.tensor_scalar_add(deni[:], den[:], 1e-30)
            rec = spool.tile([P, HH], f32, tag="rec")
            nc.vector.reciprocal(rec[:], deni[:])
            out_writer(t, agg, rec, skip_sb, opool, spool)


def _build_nc(CH):
    TOTCH = int(sum(CH))
    EP = TOTCH * P
    nc = bacc.Bacc("TRN2", target_bir_lowering=False, debug=False,
                   num_devices=NCORES)
    # --- external I/O (per core) ---
    xt = nc.dram_tensor("xt", [F_IN, NP], bf16, kind="ExternalInput")
    srcg = nc.dram_tensor("srcg", [P, TOTCH], i32, kind="ExternalInput")
    dstg = nc.dram_tensor("dstg", [P, TOTCH], i32, kind="ExternalInput")
    dstl = nc.dram_tensor("dstl", [P, TOTCH], i32, kind="ExternalInput")
    eat = nc.dram_tensor("eat", [6, EP], bf16, kind="ExternalInput")
    w1 = nc.dram_tensor("w1", [F_IN, 2056], bf16, kind="ExternalInput")
    w2 = nc.dram_tensor("w2", [D12, 2056], bf16, kind="ExternalInput")
    w3 = nc.dram_tensor("w3", [D12, D3 + NCLS + 2 * H3], bf16,
                        kind="ExternalInput")
    ve1 = nc.dram_tensor("ve1", [6, H12], bf16, kind="ExternalInput")
    ve2 = nc.dram_tensor("ve2", [6, H12], bf16, kind="ExternalInput")
    ve3 = nc.dram_tensor("ve3", [6, H3], bf16, kind="ExternalInput")
    brep1 = nc.dram_tensor("brep1", [P, D12], f32, kind="ExternalInput")
    brep2 = nc.dram_tensor("brep2", [P, D12], f32, kind="ExternalInput")
    brep3 = nc.dram_tensor("brep3", [P, NCLS], f32, kind="ExternalInput")
    outp = nc.dram_tensor("out", [NP, NCLS], f32, kind="ExternalOutput")
    # --- internal DRAM ---
    agin12 = nc.dram_tensor("agin12", [NP, RW12], bf16)
    agin3 = nc.dram_tensor("agin3", [NP, RW3], bf16)
    tbl12 = nc.dram_tensor("tbl12", [NCORES * NP, RW12], bf16,
                           addr_space="Shared")
    tbl3 = nc.dram_tensor("tbl3", [NCORES * NP, RW3], bf16,
                          addr_space="Shared")
    h1 = nc.dram_tensor("h1", [NP, D12], bf16)
    h2 = nc.dram_tensor("h2", [NP, D12], bf16)
    rg = [list(range(NCORES))]

    import contextlib
    with tile.TileContext(nc) as tc, contextlib.ExitStack() as ctx:
        cst = ctx.enter_context(tc.tile_pool(name="cst", bufs=1))
        lhsp = ctx.enter_context(tc.tile_pool(name="lhs", bufs=8))
        skp = ctx.enter_context(tc.tile_pool(name="skp", bufs=1))
        vep = ctx.enter_context(tc.tile_pool(name="vep", bufs=1))
        hp = ctx.enter_context(tc.tile_pool(name="hp", bufs=3))

        iota_row = cst.tile([P, P], i32)
        nc.gpsimd.iota(iota_row[:], pattern=[[1, P]], base=0,
                       channel_multiplier=0)
        neg1 = cst.tile([P, 1], f32)
        nc.gpsimd.memset(neg1[:], -1.0)
        srcg_sb = cst.tile([P, TOTCH], i32)
        nc.sync.dma_start(srcg_sb[:], srcg[:])
        dstg_sb = cst.tile([P, TOTCH], i32)
        nc.sync.dma_start(dstg_sb[:], dstg[:])
        dstl_sb = cst.tile([P, TOTCH], i32)
        nc.sync.dma_start(dstl_sb[:], dstl[:])
        eat_sb = cst.tile([6, EP], bf16)
        nc.sync.dma_start(eat_sb[:], eat[:])
        ve1_sb = vep.tile([6, H12], bf16, tag="v1")
        nc.sync.dma_start(ve1_sb[:], ve1[:])
        ve2_sb = vep.tile([6, H12], bf16, tag="v2")
        nc.sync.dma_start(ve2_sb[:], ve2[:])
        ve3_sb = vep.tile([6, H3], bf16, tag="v3")
        nc.sync.dma_start(ve3_sb[:], ve3[:])

        def load_lhs_from(h_dram, K):
            tiles = []
            for k in range(K):
                lt = lhsp.tile([P, NP], bf16, tag="lhs")
                nc.sync.dma_start(
                    lt[:],
                    bass.AP(h_dram, k * P, [[h_dram.shape[1], NP], [1, P]]),
                    transpose=True)
                tiles.append(lt)
            return tiles

        def make_h_writer(h_dram, Dmain, HH, CC):
            def writer(t, agg, rec, skip_sb, opool, spool):
                pre = opool.tile([P, Dmain], f32, tag="pre")
                for h in range(HH):
                    j, off = (h * CC) // 512, (h * CC) % 512
                    nc.vector.tensor_scalar_mul(
                        pre[:, h * CC:(h + 1) * CC],
                        agg[j][:, off:off + CC], rec[:, h:h + 1])
                nc.vector.tensor_tensor(
                    out=pre[:], in0=pre[:],
                    in1=skip_sb[:, t * Dmain:(t + 1) * Dmain],
                    op=mybir.AluOpType.add)
                # ELU
                expd = opool.tile([P, Dmain], f32, tag="expd")
                nc.scalar.activation(expd[:], pre[:],
                                     mybir.ActivationFunctionType.Exp)
                em1 = opool.tile([P, Dmain], f32, tag="em1")
                nc.scalar.activation(em1[:], expd[:],
                                     mybir.ActivationFunctionType.Identity,
                                     bias=neg1[:, 0:1])
                mgt = opool.tile([P, Dmain], u8, tag="mgt")
                nc.vector.tensor_scalar(mgt[:], pre[:], 0.0, None,
                                        op0=mybir.AluOpType.is_gt)
                ht = hp.tile([P, Dmain], bf16, tag="ht")
                nc.vector.select(ht[:], mgt[:], pre[:], em1[:])
                nc.sync.dma_start(h_dram[t * P:(t + 1) * P, :], ht[:])
            return writer

        def out3_writer(t, agg, rec, skip_sb, opool, spool):
            rec6 = spool.tile([P, H3], f32, tag="rec6")
            nc.vector.tensor_scalar_mul(rec6[:], rec[:], 1.0 / H3)
            acc = opool.tile([P, NCLS], f32, tag="acc3")
            nc.vector.tensor_scalar_mul(acc[:], agg[0][:, 0:NCLS],
                                        rec6[:, 0:1])
            for h in range(1, H3):
                tmp = opool.tile([P, NCLS], f32, tag="tmp3")
                nc.vector.tensor_scalar_mul(
                    tmp[:], agg[0][:, h * NCLS:(h + 1) * NCLS],
                    rec6[:, h:h + 1])
                nc.vector.tensor_tensor(out=acc[:], in0=acc[:], in1=tmp[:],
                                        op=mybir.AluOpType.add)
            nc.vector.tensor_tensor(
                out=acc[:], in0=acc[:],
                in1=skip_sb[:, t * NCLS:(t + 1) * NCLS],
                op=mybir.AluOpType.add)
            nc.sync.dma_start(outp[t * P:(t + 1) * P, :], acc[:])

        # ---------------- layer 1 ----------------
        skip_sb = skp.tile([P, NT * D12], f32, tag="sk12")
        lhs1 = []
        for k in range(F_IN // P):
            lt = lhsp.tile([P, NP], bf16, tag="lhs")
            nc.sync.dma_start(lt[:], xt[k * P:(k + 1) * P, :])
            lhs1.append(lt)
        _dense_phase(nc, tc, ctx, K=F_IN // P, wdram=w1, vesb=ve1_sb,
                     brep_dram=brep1, lhs_tiles=lhs1, skip_sb=skip_sb,
                     tbl_agin=agin12, Dmain=D12, Dskip=D12, HH=H12,
                     als_off=ALS_OFF12)
        nc.gpsimd.collective_compute(
            "AllGather", mybir.AluOpType.bypass, replica_groups=rg,
            ins=[agin12[:]], outs=[tbl12[:]])
        _edge_phase(nc, tc, ctx, CH, tbl=tbl12, vesb=ve1_sb, eat_sb=eat_sb,
                    srcg_sb=srcg_sb, dstg_sb=dstg_sb, dstl_sb=dstl_sb,
                    iota_row=iota_row, neg1=neg1, skip_sb=skip_sb,
                    Dmain=D12, HH=H12, CC=C12, als_off=ALS_OFF12,
                    ald_off=ALD_OFF12, out_writer=make_h_writer(h1, D12, H12, C12))

        # ---------------- layer 2 ----------------
        skip_sb2 = skp.tile([P, NT * D12], f32, tag="sk12")
        lhs2 = load_lhs_from(h1, D12 // P)
        _dense_phase(nc, tc, ctx, K=D12 // P, wdram=w2, vesb=ve2_sb,
                     brep_dram=brep2, lhs_tiles=lhs2, skip_sb=skip_sb2,
                     tbl_agin=agin12, Dmain=D12, Dskip=D12, HH=H12,
                     als_off=ALS_OFF12)
        nc.gpsimd.collective_compute(
            "AllGather", mybir.AluOpType.bypass, replica_groups=rg,
            ins=[agin12[:]], outs=[tbl12[:]])
        _edge_phase(nc, tc, ctx, CH, tbl=tbl12, vesb=ve2_sb, eat_sb=eat_sb,
                    srcg_sb=srcg_sb, dstg_sb=dstg_sb, dstl_sb=dstl_sb,
                    iota_row=iota_row, neg1=neg1, skip_sb=skip_sb2,
                    Dmain=D12, HH=H12, CC=C12, als_off=ALS_OFF12,
                    ald_off=ALD_OFF12, out_writer=make_h_writer(h2, D12, H12, C12))

        # ---------------- layer 3 ----------------
        skip_sb3 = skp.tile([P, NT * NCLS], f32, tag="sk3")
        lhs3 = load_lhs_from(h2, D12 // P)
        _dense_phase(nc, tc, ctx, K=D12 // P, wdram=w3, vesb=ve3_sb,
                     brep_dram=brep3, lhs_tiles=lhs3, skip_sb=skip_sb3,
                     tbl_agin=agin3, Dmain=D3, Dskip=NCLS, HH=H3,
                     als_off=ALS_OFF3)
        nc.gpsimd.collective_compute(
            "AllGather", mybir.AluOpType.bypass, replica_groups=rg,
            ins=[agin3[:]], outs=[tbl3[:]])
        _edge_phase(nc, tc, ctx, CH, tbl=tbl3, vesb=ve3_sb, eat_sb=eat_sb,
                    srcg_sb=srcg_sb, dstg_sb=dstg_sb, dstl_sb=dstl_sb,
                    iota_row=iota_row, neg1=neg1, skip_sb=skip_sb3,
                    Dmain=D3, HH=H3, CC=C3, als_off=ALS_OFF3,
                    ald_off=ALD_OFF3, out_writer=out3_writer)

    nc.compile()
    return nc


@functools.lru_cache(maxsize=2)
def _built(CH_tuple):
    return _build_nc(list(CH_tuple))


# ------------------------------------------------------------------ host prep

def _fold_as(W, a):
    H, C = a.shape
    return np.einsum('dhc,hc->dh', W.reshape(W.shape[0], H, C), a)


def _prep(inputs):
    x = np.asarray(inputs['x'], np.float32)
    ei = np.asarray(inputs['edge_index'], np.int64)
    ea = np.asarray(inputs['edge_attr'], np.float32)
    src, dst = ei[0], ei[1]

    order = np.argsort(dst, kind='stable')
    src, dst = src[order], dst[order]
    ea_s = ea[order]

    core_of = dst // NSH
    loc = dst - core_of * NSH
    tile_of = loc // P

    # chunk counts per dst-tile slot (max over cores)
    cnt = np.zeros((NCORES, NT), np.int64)
    for c in range(NCORES):
        m = core_of == c
        cnt[c] = np.bincount(tile_of[m], minlength=NT)
    CH = tuple(int(v) for v in np.ceil(cnt.max(axis=0) / P).astype(np.int64))
    TOTCH = int(sum(CH))
    EP = TOTCH * P

    pos = (src // NSH) * NP + (src % NSH)      # AG-table row of src node
    posd = (dst // NSH) * NP + (dst % NSH)

    per_core = []
    for c in range(NCORES):
        sg = np.zeros(EP, np.int32)
        dg = np.zeros(EP, np.int32)
        dl = np.full(EP, -1, np.int32)
        et = np.zeros((EP, 6), np.float32)
        off = 0
        m = core_of == c
        for t in range(NT):
            mt = m & (tile_of == t)
            k = int(mt.sum())
            sl = slice(off, off + k)
            sg[sl] = pos[mt]
            dg[sl] = posd[mt]
            dl[sl] = loc[mt] - t * P
            et[sl] = ea_s[mt]
            off += CH[t] * P
        tr = lambda a: np.ascontiguousarray(a.reshape(TOTCH, P).T)
        xt = np.zeros((F_IN, NP), np.float32)
        xt[:, :NSH] = x[c * NSH:(c + 1) * NSH].T
        per_core.append(dict(
            xt=xt.astype(BF),
            srcg=tr(sg), dstg=tr(dg), dstl=tr(dl),
            eat=np.ascontiguousarray(et.T).astype(BF),
        ))

    def wext(W, Wl, a_s, a_d):
        return np.concatenate(
            [W, Wl, _fold_as(W, a_s), _fold_as(W, a_d)], axis=1).astype(BF)

    i = {k: np.asarray(v, np.float32) for k, v in inputs.items()
         if k not in ('edge_index',)}
    shared = dict(
        w1=wext(i['W1'], i['Wl1'], i['as1'], i['ad1']),
        w2=wext(i['W2'], i['Wl2'], i['as2'], i['ad2']),
        w3=wext(i['W3'], i['Wl3'], i['as3'], i['ad3']),
        ve1=_fold_as(i['We1'], i['ae1']).astype(BF),
        ve2=_fold_as(i['We2'], i['ae2']).astype(BF),
        ve3=_fold_as(i['We3'], i['ae3']).astype(BF),
        brep1=np.repeat((i['b1'] + i['bl1'])[None, :], P, 0).astype(np.float32),
        brep2=np.repeat((i['b2'] + i['bl2'])[None, :], P, 0).astype(np.float32),
        brep3=np.repeat((i['b3'] + i['bl3'])[None, :], P, 0).astype(np.float32),
    )
    in_maps = [dict(per_core[c], **shared) for c in range(NCORES)]
    return CH, in_maps


def kernel(**inputs):
    CH, in_maps = _prep(inputs)
    nc = _built(CH)
    res = run_bass_kernel_spmd(nc, in_maps, core_ids=list(range(NCORES)))
    out = np.empty((N, NCLS), np.float32)
    for c in range(NCORES):
        out[c * NSH:(c + 1) * NSH] = res.results[c]["out"][:NSH]
    return out


if __name__ == "__main__":
    inp = np.load('/tmp/inputs.npy', allow_pickle=True).item()
    got = kernel(**inp)
    ref = np.load('/tmp/ref_out.npy')
    d = np.abs(got - ref)
    s = np.abs(ref).max()
    print("absmax err:", d.max(), "scale:", s, "rel:", d.max() / s)


# revision 11
# speedup vs baseline: 78.2397x; 78.2397x over previous
"""3-layer GAT (GATConv) network on 8 Trainium2 NeuronCores.

Strategy (edge-parallel, dst-sharded):
  - Nodes are sharded evenly: core c owns nodes [c*1250, (c+1)*1250), padded
    to NP=1280 (10 tiles of 128).  Edges are sorted by dst and assigned to the
    core owning their dst node, grouped per 128-node dst tile, padded to a
    per-tile chunk count CH[t] shared across cores (same NEFF everywhere).
  - Per layer: each core computes the dense transforms for its own node shard
    (fused matmul [W | Wl | W@As | W@Ad]), packs a per-node "table row"
    [Hfeat bf16 | ALs fp32 | ALd fp32 | pad] and AllGathers the table.  The
    edge phase gathers table rows by src (GPSIMD dma_gather, one op per
    half-dst-tile), builds per-edge softmax numerators
    ex = exp(leaky_relu(ALs[src]+ALd[dst]+ALe)), scales the gathered features
    by ex, and aggregates per dst tile with a one-hot mask matmul (which also
    produces the softmax denominators).  Output is normalized, skip+bias
    added, ELU applied, and becomes the next layer's input.  Inter-core
    traffic is only the 3 table AllGathers.
"""
import functools

import numpy as np
import ml_dtypes

import concourse.bass as bass
import concourse.bacc as bacc
import concourse.tile as tile
from concourse import mybir
from concourse.bass_utils import run_bass_kernel_spmd
from concourse.library_config import mlp

P = 128
NCORES = 8
N = 10000
E = 160000
NSH = N // NCORES          # 1250 real nodes per core
NT = 10                    # dst tiles per core
NP = NT * P                # padded nodes per core (1280)
NTBL = NCORES * NP         # allgathered table rows
F_IN = 512
D12 = 1024                 # hidden width layers 1-2
H12, C12 = 4, 256
D3, H3, C3 = 96, 6, 16
NCLS = 16
RW12 = 1152                # table row lanes (bf16), 2304 B (256B multiple)
RW3 = 128                  # layer 3 row: 96 feat + 12 ALs + 12 ALd + 8 pad

f32 = mybir.dt.float32
bf16 = mybir.dt.bfloat16
i32 = mybir.dt.int32
i16 = mybir.dt.int16
u8 = mybir.dt.uint8
BF = ml_dtypes.bfloat16


# ----------------------------------------------------------------- device code

def _dense_phase(nc, tc, *, K, wdram, brep_dram, lhs_tiles,
                 skip_sb, tbl_agin, Dmain, Dskip, HH, rw, als_lane):
    """Dense transforms for own node shard.  Writes table rows (Hfeat+AL)
    into tbl_agin and skip+bias into skip_sb ([P, NT*Dskip] fp32)."""
    import contextlib
    Wcols = Dmain + Dskip + 2 * HH
    ms = Dmain + Dskip               # main+skip region
    nj = (ms + 511) // 512
    with contextlib.ExitStack() as st:
        wpool = st.enter_context(tc.tile_pool(name="wp", bufs=K))
        pspool = st.enter_context(tc.tile_pool(name="dps", bufs=4, space="PSUM"))
        alpool = st.enter_context(tc.tile_pool(name="dal", bufs=2, space="PSUM"))
        tpool = st.enter_context(tc.tile_pool(name="dtab", bufs=3))
        bpool = st.enter_context(tc.tile_pool(name="dbr", bufs=1))

        brep = bpool.tile([P, Dskip], f32)
        nc.sync.dma_start(brep[:], brep_dram[:])
        wsb = []
        for k in range(K):
            w = wpool.tile([P, Wcols], bf16, tag="w")
            nc.sync.dma_start(w[:], wdram[k * P:(k + 1) * P, :])
            wsb.append(w)

        for t in range(NT):
            ps = []
            for j in range(nj):
                width = min(512, ms - j * 512)
                ps.append(pspool.tile([P, width], f32, space="PSUM", tag="m",
                                      name=f"dps{j}"))
            psal = alpool.tile([P, 2 * HH], f32, space="PSUM", tag="al")
            for k in range(K):
                lhsT = lhs_tiles[k][:, t * P:(t + 1) * P]
                first, last = (k == 0), (k == K - 1)
                for j in range(nj):
                    width = min(512, ms - j * 512)
                    nc.tensor.matmul(ps[j][:], lhsT=lhsT,
                                     rhs=wsb[k][:, j * 512:j * 512 + width],
                                     start=first, stop=last)
                nc.tensor.matmul(psal[:], lhsT=lhsT,
                                 rhs=wsb[k][:, ms:ms + 2 * HH],
                                 start=first, stop=last)
            # pack table row: Hfeat bf16 + AL fp32 (bitcast into bf16 lanes)
            tabt = tpool.tile([P, rw], bf16, tag="t")
            done = 0
            while done < Dmain:
                j, off = done // 512, done % 512
                width = min(512 - off, Dmain - done)
                nc.scalar.activation(
                    tabt[:, done:done + width], ps[j][:, off:off + width],
                    mybir.ActivationFunctionType.Copy)
                done += width
            nc.vector.tensor_copy(
                tabt[:, als_lane:als_lane + 4 * HH].bitcast(f32), psal[:])
            nc.sync.dma_start(tbl_agin[t * P:(t + 1) * P, :], tabt[:])
            # skip + bias -> persistent sbuf
            done = 0
            while done < Dskip:
                j, off = (Dmain + done) // 512, (Dmain + done) % 512
                width = min(512 - off, Dskip - done)
                nc.vector.tensor_tensor(
                    out=skip_sb[:, t * Dskip + done:t * Dskip + done + width],
                    in0=ps[j][:, off:off + width],
                    in1=brep[:, done:done + width],
                    op=mybir.AluOpType.add)
                done += width


def _edge_phase(nc, tc, CH, *, tbl, vesb, eat, src16_sb, dst16_sb,
                dstl_sb, iota_row, skip_sb, Dmain, HH, CC, rw,
                als_f0, ald_f0, a_off, out_writer):
    """Edge gather + softmax + aggregation per dst tile."""
    import contextlib
    with contextlib.ExitStack() as st:
        gpool = st.enter_context(tc.tile_pool(name="eg", bufs=2))
        gppool = st.enter_context(tc.tile_pool(name="egp", bufs=4))
        spool = st.enter_context(tc.tile_pool(name="esm", bufs=4))
        mpool = st.enter_context(tc.tile_pool(name="emask", bufs=4))
        eatp = st.enter_context(tc.tile_pool(name="eeat", bufs=2))
        apool = st.enter_context(tc.tile_pool(name="eagg", bufs=4, space="PSUM"))
        epool = st.enter_context(tc.tile_pool(name="eale", bufs=2, space="PSUM"))
        opool = st.enter_context(tc.tile_pool(name="eout", bufs=2))

        nja = (Dmain + 511) // 512
        gofs = 0  # global chunk offset
        for t in range(NT):
            cht = CH[t]
            eat_t = eatp.tile([6, cht * P], bf16, tag="eat")
            nc.sync.dma_start(eat_t[:], eat[:, gofs * P:(gofs + cht) * P])
            agg = []
            for j in range(nja):
                width = min(512, Dmain - j * 512)
                agg.append(apool.tile([P, width], f32, space="PSUM",
                                      tag="agg", name=f"agg{j}"))
            den = epool.tile([P, HH], f32, space="PSUM", tag="den")
            GMAX = 6  # SWDGE descriptor ring holds ~1024 rows; 768/op is safe
            for j0 in range(0, cht, GMAX):
                gk = min(GMAX, cht - j0)
                jg = gofs + j0
                # one gather op per group: full rows by src
                G = gpool.tile([P, gk * rw], bf16, tag="G")
                nc.gpsimd.dma_gather(
                    out_ap=G[:].rearrange("p (k r) -> p k r", k=gk),
                    in_ap=bass.AP(tbl, 0, [[rw, NTBL], [1, rw]]),
                    idxs_ap=src16_sb[:, jg * 8:(jg + gk) * 8],
                    num_idxs=gk * P, num_idxs_reg=gk * P,
                    elem_size=rw, elem_step=rw)
                # AL region by dst (128 lanes at a_off)
                A = gpool.tile([P, gk * 128], bf16, tag="A")
                nc.gpsimd.dma_gather(
                    out_ap=A[:].rearrange("p (k r) -> p k r", k=gk),
                    in_ap=bass.AP(tbl, a_off, [[rw, NTBL], [1, 128]]),
                    idxs_ap=dst16_sb[:, jg * 8:(jg + gk) * 8],
                    num_idxs=gk * P, num_idxs_reg=gk * P,
                    elem_size=128, elem_step=rw)
                # ALe for the group (one small matmul per chunk, shared psum)
                psale = epool.tile([P, gk * HH], f32, space="PSUM", tag="ale")
                for j in range(gk):
                    nc.tensor.matmul(
                        psale[:, j * HH:(j + 1) * HH],
                        lhsT=eat_t[:, (j0 + j) * P:(j0 + j + 1) * P],
                        rhs=vesb[:], start=True, stop=True)
                # logits -> leaky -> exp, batched over the group
                Gf = G[:].bitcast(f32).rearrange("p (k r) -> p k r", k=gk)
                Af = A[:].bitcast(f32).rearrange("p (k r) -> p k r", k=gk)
                lg = spool.tile([P, gk * HH], f32, tag="lg")
                lg3 = lg[:].rearrange("p (k r) -> p k r", k=gk)
                nc.vector.tensor_tensor(
                    out=lg3, in0=Gf[:, :, als_f0:als_f0 + HH],
                    in1=Af[:, :, ald_f0:ald_f0 + HH], op=mybir.AluOpType.add)
                nc.vector.tensor_tensor(out=lg[:], in0=lg[:], in1=psale[:],
                                        op=mybir.AluOpType.add)
                lk = spool.tile([P, gk * HH], f32, tag="lk")
                nc.vector.tensor_scalar_mul(lk[:], lg[:], 0.2)
                nc.vector.tensor_tensor(out=lk[:], in0=lg[:], in1=lk[:],
                                        op=mybir.AluOpType.max)
                ex = spool.tile([P, gk * HH], f32, tag="ex")
                nc.scalar.activation(ex[:], lk[:],
                                     mybir.ActivationFunctionType.Exp)
                exb = spool.tile([P, gk * HH], bf16, tag="exb")
                nc.scalar.activation(exb[:], ex[:],
                                     mybir.ActivationFunctionType.Copy)
                # per-chunk: mask, scaled features, aggregation matmuls
                for j in range(gk):
                    ch = j0 + j
                    first, last = (ch == 0), (ch == cht - 1)
                    mask = mpool.tile([P, P], bf16, tag="mk")
                    nc.vector.tensor_tensor(
                        out=mask[:],
                        in0=dstl_sb[:, jg + j:jg + j + 1].to_broadcast([P, P]),
                        in1=iota_row[:], op=mybir.AluOpType.is_equal)
                    Gp = gppool.tile([P, Dmain + HH], bf16, tag="Gp")
                    if CC >= 128:
                        for h in range(HH):
                            nc.vector.tensor_scalar_mul(
                                Gp[:, h * CC:(h + 1) * CC],
                                G[:, j * rw + h * CC:j * rw + (h + 1) * CC],
                                ex[:, j * HH + h:j * HH + h + 1])
                    else:  # layer 3: one multiply with head-broadcast ex
                        exs = exb[:, j * HH:(j + 1) * HH]
                        exbc = bass.AP(exs.tensor, exs.offset,
                                       [exs.ap[0], [1, HH], [0, CC]])
                        nc.vector.tensor_tensor(
                            out=Gp[:, 0:Dmain].rearrange(
                                "p (h c) -> p h c", h=HH),
                            in0=G[:, j * rw:j * rw + Dmain].rearrange(
                                "p (h c) -> p h c", h=HH),
                            in1=exbc, op=mybir.AluOpType.mult)
                    nc.vector.tensor_copy(Gp[:, Dmain:Dmain + HH],
                                          exb[:, j * HH:(j + 1) * HH])
                    for jj in range(nja):
                        width = min(512, Dmain - jj * 512)
                        nc.tensor.matmul(agg[jj][:], lhsT=mask[:],
                                         rhs=Gp[:, jj * 512:jj * 512 + width],
                                         start=first, stop=last)
                    nc.tensor.matmul(den[:], lhsT=mask[:],
                                     rhs=Gp[:, Dmain:Dmain + HH],
                                     start=first, stop=last)
            gofs += cht
            # normalize + skip
            deni = spool.tile([P, HH], f32, tag="deni")
            nc.vector.tensor_scalar_add(deni[:], den[:], 1e-30)
            rec = spool.tile([P, HH], f32, tag="rec")
            nc.vector.reciprocal(rec[:], deni[:])
            out_writer(t, agg, rec, skip_sb, opool, spool)


def _build_nc(CH, use_collectives=True):
    TOTCH = int(sum(CH))
    EP = TOTCH * P
    nc = bacc.Bacc("TRN2", target_bir_lowering=False, debug=False,
                   num_devices=NCORES)
    # --- external I/O (per core) ---
    xt = nc.dram_tensor("xt", [F_IN, NP], bf16, kind="ExternalInput")
    src16 = nc.dram_tensor("src16", [P, EP // 16], i16, kind="ExternalInput")
    dst16 = nc.dram_tensor("dst16", [P, EP // 16], i16, kind="ExternalInput")
    dstl = nc.dram_tensor("dstl", [P, TOTCH], bf16, kind="ExternalInput")
    eat = nc.dram_tensor("eat", [6, EP], bf16, kind="ExternalInput")
    w1 = nc.dram_tensor("w1", [F_IN, 2056], bf16, kind="ExternalInput")
    w2 = nc.dram_tensor("w2", [D12, 2056], bf16, kind="ExternalInput")
    w3 = nc.dram_tensor("w3", [D12, D3 + NCLS + 2 * H3], bf16,
                        kind="ExternalInput")
    ve1 = nc.dram_tensor("ve1", [6, H12], bf16, kind="ExternalInput")
    ve2 = nc.dram_tensor("ve2", [6, H12], bf16, kind="ExternalInput")
    ve3 = nc.dram_tensor("ve3", [6, H3], bf16, kind="ExternalInput")
    brep1 = nc.dram_tensor("brep1", [P, D12], f32, kind="ExternalInput")
    brep2 = nc.dram_tensor("brep2", [P, D12], f32, kind="ExternalInput")
    brep3 = nc.dram_tensor("brep3", [P, NCLS], f32, kind="ExternalInput")
    outp = nc.dram_tensor("out", [NP, NCLS], f32, kind="ExternalOutput")
    # --- internal DRAM ---
    agin12 = nc.dram_tensor("agin12", [NP, RW12], bf16)
    agin3 = nc.dram_tensor("agin3", [NP, RW3], bf16)
    tbl12 = nc.dram_tensor("tbl12", [NTBL, RW12], bf16, addr_space="Shared")
    tbl3 = nc.dram_tensor("tbl3", [NTBL, RW3], bf16, addr_space="Shared")
    h1 = nc.dram_tensor("h1", [NP, D12], bf16)
    h2 = nc.dram_tensor("h2", [NP, D12], bf16)
    rg = [list(range(NCORES))]

    import contextlib
    with tile.TileContext(nc) as tc, contextlib.ExitStack() as ctx:
        cst = ctx.enter_context(tc.tile_pool(name="cst", bufs=1))
        lhsp = ctx.enter_context(tc.tile_pool(name="lhs", bufs=8))
        skp = ctx.enter_context(tc.tile_pool(name="skp", bufs=1))
        vep = ctx.enter_context(tc.tile_pool(name="vep", bufs=1))
        hp = ctx.enter_context(tc.tile_pool(name="hp", bufs=3))

        nc.gpsimd.load_library(mlp)
        iota_row = cst.tile([P, P], bf16)
        nc.gpsimd.iota(iota_row[:], pattern=[[1, P]], base=0,
                       channel_multiplier=0,
                       allow_small_or_imprecise_dtypes=True)
        neg1 = cst.tile([P, 1], f32)
        nc.gpsimd.memset(neg1[:], -1.0)
        src16_sb = cst.tile([P, EP // 16], i16)
        nc.sync.dma_start(src16_sb[:], src16[:])
        dst16_sb = cst.tile([P, EP // 16], i16)
        nc.sync.dma_start(dst16_sb[:], dst16[:])
        dstl_sb = cst.tile([P, TOTCH], bf16)
        nc.sync.dma_start(dstl_sb[:], dstl[:])
        ve1_sb = vep.tile([6, H12], bf16, tag="v1")
        nc.sync.dma_start(ve1_sb[:], ve1[:])
        ve2_sb = vep.tile([6, H12], bf16, tag="v2")
        nc.sync.dma_start(ve2_sb[:], ve2[:])
        ve3_sb = vep.tile([6, H3], bf16, tag="v3")
        nc.sync.dma_start(ve3_sb[:], ve3[:])

        def allgather(agin, tbl):
            if use_collectives:
                nc.gpsimd.collective_compute(
                    "AllGather", mybir.AluOpType.bypass, replica_groups=rg,
                    ins=[agin[:]], outs=[tbl[:]])
            else:  # timing-sim stand-in: 8 local copies approximating AG traffic
                for c in range(NCORES):
                    nc.sync.dma_start(tbl[c * NP:(c + 1) * NP, :], agin[:])

        def load_lhs_from(h_dram, K):
            tiles = []
            for k in range(K):
                lt = lhsp.tile([P, NP], bf16, tag="lhs")
                nc.sync.dma_start(
                    lt[:],
                    bass.AP(h_dram, k * P, [[h_dram.shape[1], NP], [1, P]]),
                    transpose=True)
                tiles.append(lt)
            return tiles

        def make_h_writer(h_dram):
            def writer(t, agg, rec, skip_sb, opool, spool):
                pre = opool.tile([P, D12], f32, tag="pre")
                for h in range(H12):
                    j, off = (h * C12) // 512, (h * C12) % 512
                    nc.vector.tensor_scalar_mul(
                        pre[:, h * C12:(h + 1) * C12],
                        agg[j][:, off:off + C12], rec[:, h:h + 1])
                nc.vector.tensor_tensor(
                    out=pre[:], in0=pre[:],
                    in1=skip_sb[:, t * D12:(t + 1) * D12],
                    op=mybir.AluOpType.add)
                # ELU
                expd = opool.tile([P, D12], f32, tag="expd")
                nc.scalar.activation(expd[:], pre[:],
                                     mybir.ActivationFunctionType.Exp)
                em1 = opool.tile([P, D12], f32, tag="em1")
                nc.scalar.activation(em1[:], expd[:],
                                     mybir.ActivationFunctionType.Identity,
                                     bias=neg1[:, 0:1])
                mgt = opool.tile([P, D12], u8, tag="mgt")
                nc.vector.tensor_scalar(mgt[:], pre[:], 0.0, None,
                                        op0=mybir.AluOpType.is_gt)
                ht = hp.tile([P, D12], bf16, tag="ht")
                nc.vector.select(ht[:], mgt[:], pre[:], em1[:])
                nc.sync.dma_start(h_dram[t * P:(t + 1) * P, :], ht[:])
            return writer

        def out3_writer(t, agg, rec, skip_sb, opool, spool):
            rec6 = spool.tile([P, H3], f32, tag="rec6")
            nc.vector.tensor_scalar_mul(rec6[:], rec[:], 1.0 / H3)
            acc = opool.tile([P, NCLS], f32, tag="acc3")
            nc.vector.tensor_scalar_mul(acc[:], agg[0][:, 0:NCLS],
                                        rec6[:, 0:1])
            for h in range(1, H3):
                tmp = opool.tile([P, NCLS], f32, tag="tmp3")
                nc.vector.tensor_scalar_mul(
                    tmp[:], agg[0][:, h * NCLS:(h + 1) * NCLS],
                    rec6[:, h:h + 1])
                nc.vector.tensor_tensor(out=acc[:], in0=acc[:], in1=tmp[:],
                                        op=mybir.AluOpType.add)
            nc.vector.tensor_tensor(
                out=acc[:], in0=acc[:],
                in1=skip_sb[:, t * NCLS:(t + 1) * NCLS],
                op=mybir.AluOpType.add)
            nc.sync.dma_start(outp[t * P:(t + 1) * P, :], acc[:])

        # ---------------- layer 1 ----------------
        skip_sb = skp.tile([P, NT * D12], f32, tag="sk12")
        lhs1 = []
        for k in range(F_IN // P):
            lt = lhsp.tile([P, NP], bf16, tag="lhs")
            nc.sync.dma_start(lt[:], xt[k * P:(k + 1) * P, :])
            lhs1.append(lt)
        _dense_phase(nc, tc, K=F_IN // P, wdram=w1, brep_dram=brep1,
                     lhs_tiles=lhs1, skip_sb=skip_sb, tbl_agin=agin12,
                     Dmain=D12, Dskip=D12, HH=H12, rw=RW12, als_lane=1024)
        allgather(agin12, tbl12)
        _edge_phase(nc, tc, CH, tbl=tbl12, vesb=ve1_sb, eat=eat,
                    src16_sb=src16_sb, dst16_sb=dst16_sb, dstl_sb=dstl_sb,
                    iota_row=iota_row, skip_sb=skip_sb,
                    Dmain=D12, HH=H12, CC=C12, rw=RW12,
                    als_f0=512, ald_f0=4, a_off=1024,
                    out_writer=make_h_writer(h1))

        # ---------------- layer 2 ----------------
        skip_sb2 = skp.tile([P, NT * D12], f32, tag="sk12")
        lhs2 = load_lhs_from(h1, D12 // P)
        _dense_phase(nc, tc, K=D12 // P, wdram=w2, brep_dram=brep2,
                     lhs_tiles=lhs2, skip_sb=skip_sb2, tbl_agin=agin12,
                     Dmain=D12, Dskip=D12, HH=H12, rw=RW12, als_lane=1024)
        allgather(agin12, tbl12)
        _edge_phase(nc, tc, CH, tbl=tbl12, vesb=ve2_sb, eat=eat,
                    src16_sb=src16_sb, dst16_sb=dst16_sb, dstl_sb=dstl_sb,
                    iota_row=iota_row, skip_sb=skip_sb2,
                    Dmain=D12, HH=H12, CC=C12, rw=RW12,
                    als_f0=512, ald_f0=4, a_off=1024,
                    out_writer=make_h_writer(h2))

        # ---------------- layer 3 ----------------
        skip_sb3 = skp.tile([P, NT * NCLS], f32, tag="sk3")
        lhs3 = load_lhs_from(h2, D12 // P)
        _dense_phase(nc, tc, K=D12 // P, wdram=w3, brep_dram=brep3,
                     lhs_tiles=lhs3, skip_sb=skip_sb3, tbl_agin=agin3,
                     Dmain=D3, Dskip=NCLS, HH=H3, rw=RW3, als_lane=96)
        allgather(agin3, tbl3)
        _edge_phase(nc, tc, CH, tbl=tbl3, vesb=ve3_sb, eat=eat,
                    src16_sb=src16_sb, dst16_sb=dst16_sb, dstl_sb=dstl_sb,
                    iota_row=iota_row, skip_sb=skip_sb3,
                    Dmain=D3, HH=H3, CC=C3, rw=RW3,
                    als_f0=48, ald_f0=54, a_off=0,
                    out_writer=out3_writer)

    nc.compile()
    return nc


@functools.lru_cache(maxsize=2)
def _built(CH_tuple):
    return _build_nc(list(CH_tuple))


# ------------------------------------------------------------------ host prep

def _fold_as(W, a):
    H, C = a.shape
    return np.einsum('dhc,hc->dh', W.reshape(W.shape[0], H, C), a)


def _prep(inputs):
    x = np.asarray(inputs['x'], np.float32)
    ei = np.asarray(inputs['edge_index'], np.int64)
    ea = np.asarray(inputs['edge_attr'], np.float32)
    src, dst = ei[0], ei[1]

    order = np.argsort(dst, kind='stable')
    src, dst = src[order], dst[order]
    ea_s = ea[order]

    core_of = dst // NSH
    loc = dst - core_of * NSH
    tile_of = loc // P

    cnt = np.zeros((NCORES, NT), np.int64)
    for c in range(NCORES):
        m = core_of == c
        cnt[c] = np.bincount(tile_of[m], minlength=NT)
    CH = tuple(int(v) for v in np.ceil(cnt.max(axis=0) / P).astype(np.int64))
    TOTCH = int(sum(CH))
    EP = TOTCH * P

    pos = (src // NSH) * NP + (src % NSH)      # AG-table row of src node
    posd = (dst // NSH) * NP + (dst % NSH)

    def wrap16(a):  # flat [EP] -> [128, EP//16] int16 (16-wrap, 8x replicate)
        return np.tile(np.ascontiguousarray(a.reshape(-1, 16).T), (8, 1)
                       ).astype(np.int16)

    per_core = []
    for c in range(NCORES):
        sg = np.zeros(EP, np.int64)
        dg = np.zeros(EP, np.int64)
        dl = np.full(EP, -1, np.int64)
        et = np.zeros((EP, 6), np.float32)
        off = 0
        m = core_of == c
        for t in range(NT):
            mt = m & (tile_of == t)
            k = int(mt.sum())
            sl = slice(off, off + k)
            sg[sl] = pos[mt]
            dg[sl] = posd[mt]
            dl[sl] = loc[mt] - t * P
            et[sl] = ea_s[mt]
            off += CH[t] * P
        xt = np.zeros((F_IN, NP), np.float32)
        xt[:, :NSH] = x[c * NSH:(c + 1) * NSH].T
        per_core.append(dict(
            xt=xt.astype(BF),
            src16=wrap16(sg), dst16=wrap16(dg),
            dstl=np.ascontiguousarray(
                dl.reshape(TOTCH, P).T).astype(BF),
            eat=np.ascontiguousarray(et.T).astype(BF),
        ))

    def wext(W, Wl, a_s, a_d):
        return np.concatenate(
            [W, Wl, _fold_as(W, a_s), _fold_as(W, a_d)], axis=1).astype(BF)

    i = {k: np.asarray(v, np.float32) for k, v in inputs.items()
         if k not in ('edge_index',)}
    shared = dict(
        w1=wext(i['W1'], i['Wl1'], i['as1'], i['ad1']),
        w2=wext(i['W2'], i['Wl2'], i['as2'], i['ad2']),
        w3=wext(i['W3'], i['Wl3'], i['as3'], i['ad3']),
        ve1=_fold_as(i['We1'], i['ae1']).astype(BF),
        ve2=_fold_as(i['We2'], i['ae2']).astype(BF),
        ve3=_fold_as(i['We3'], i['ae3']).astype(BF),
        brep1=np.repeat((i['b1'] + i['bl1'])[None, :], P, 0).astype(np.float32),
        brep2=np.repeat((i['b2'] + i['bl2'])[None, :], P, 0).astype(np.float32),
        brep3=np.repeat((i['b3'] + i['bl3'])[None, :], P, 0).astype(np.float32),
    )
    in_maps = [dict(per_core[c], **shared) for c in range(NCORES)]
    return CH, in_maps


def kernel(**inputs):
    CH, in_maps = _prep(inputs)
    nc = _built(CH)
    res = run_bass_kernel_spmd(nc, in_maps, core_ids=list(range(NCORES)))
    out = np.empty((N, NCLS), np.float32)
    for c in range(NCORES):
        out[c * NSH:(c + 1) * NSH] = res.results[c]["out"][:NSH]
    return out


if __name__ == "__main__":
    inp = np.load('/tmp/inputs.npy', allow_pickle=True).item()
    got = kernel(**inp)
    ref = np.load('/tmp/ref_out.npy')
    d = np.abs(got - ref)
    s = np.abs(ref).max()
    print("absmax err:", d.max(), "scale:", s, "rel:", d.max() / s)
